# revision 11
# baseline (speedup 1.0000x reference)
"""Differential attention kernel for 8 Trainium2 NeuronCores.

Reference computation (per batch b, output head h, with score heads 2h, 2h+1):
    S_i = q[b,2h+i] @ k[b,2h+i].T * (1/8), causal-masked, softmax -> P_i
    y[b,h] = RMSNorm(P_1 @ v - lambda_h * P_2 @ v) * (1 - lambda_init)

Sharding: the 64 (b, h) head-pairs are split 8 per core (data + head parallel).
Lambda params / rms weight are replicated (lambda reduced host-side to the
per-head scalar the reference computes).

Device algorithm per head-pair (T=1024, d=64, vd=128; 128-row tiles):
  - scores computed TRANSPOSED: S^T[s, q] = k~.T @ q~ with k~, q~ = [64, T]
    d-major operands (host supplies q/k pre-transposed, packed as one
    [128, T] fp16 tile per score head: rows 0:64 = q~, rows 64:128 = k~).
    P~^T[s, q] is then directly the stationary operand of the PV matmul.
  - exp on ACT with scale=1/8 fused; unnormalized (|S|*scale <= ~8, safe fp32).
  - causal handling structural: only s-tiles j <= q-tile i computed; the
    diagonal block is masked AFTER exp by one gpsimd affine_select covering
    both heads.
  - V tiles carry an appended ones-column (built host-side), so PV yields the
    softmax denominators in column 128 of each Y.
  - z = Y1 - (lam*s1/s2) * Y2; RMSNorm is scale-invariant per row, so
    normalize z directly.
  - rsqrt for RMSNorm via fast-inverse-sqrt on DVE (bit seed + 2 Newton
    steps) -> no ACT table switches; finalize is per-pair and overlaps the
    remaining pairs' compute (kills the serial tail of v1).
  - all input DMAs are issued up-front in priority order with host-prepared
    contiguous layouts (2KB+ per-partition lines; no gather descriptors);
    output is written in kernel layout [BLK, NJ, BLK] and untransposed on
    host, so the out DMA is contiguous too.
"""

import contextlib
import ctypes
import math
import sys
import types
from contextlib import ExitStack

if "/opt/trn_rl_repo" not in sys.path:
    sys.path.insert(0, "/opt/trn_rl_repo")

import numpy as np


# ---------------------------------------------------------------------------
# antenv.axon_hooks shim: the agent image's antenv lacks axon_hooks, which
# concourse.bass_utils hard-imports when trace=True under axon. Recreate the
# module and register the same ctypes NTFF hook trn_boot would have.
def _install_axon_ntff_shim():
    if "antenv.axon_hooks" in sys.modules:
        return
    mod = types.ModuleType("antenv.axon_hooks")
    mod._hook = None
    mod.set_axon_ntff_profile_hook = lambda h: setattr(mod, "_hook", h)
    mod.get_axon_ntff_profile_hook = lambda: mod._hook
    sys.modules["antenv.axon_hooks"] = mod
    try:
        import antenv

        antenv.axon_hooks = mod
    except ImportError:
        pass
    try:
        lib = ctypes.CDLL("/opt/axon/libaxon_pjrt.so")
    except OSError:
        return
    if not hasattr(lib, "axon_start_nrt_profile"):
        return
    lib.axon_start_nrt_profile.argtypes = [
        ctypes.POINTER(ctypes.c_int64),
        ctypes.c_size_t,
    ]
    lib.axon_start_nrt_profile.restype = ctypes.c_int64
    lib.axon_stop_nrt_profile.argtypes = [ctypes.c_char_p]
    lib.axon_stop_nrt_profile.restype = ctypes.c_int64

    @contextlib.contextmanager
    def _hook(output_dir, device_ids):
        import jax

        jax.devices()
        if device_ids:
            ids = (ctypes.c_int64 * len(device_ids))(*device_ids)
            rc = lib.axon_start_nrt_profile(ids, len(device_ids))
        else:
            rc = lib.axon_start_nrt_profile(None, 0)
        if rc != 0:
            raise RuntimeError(f"axon_start_nrt_profile rc={rc}")
        try:
            yield
        finally:
            n = lib.axon_stop_nrt_profile(str(output_dir).encode())
            if n < 0:
                raise RuntimeError(f"axon_stop_nrt_profile rc={n}")

    mod.set_axon_ntff_profile_hook(_hook)


_install_axon_ntff_shim()

import concourse.bass as bass  # noqa: E402
import concourse.mybir as mybir  # noqa: E402
import concourse.tile as tile  # noqa: E402
from concourse import bacc, bass_utils  # noqa: E402
from concourse.alu_op_type import AluOpType  # noqa: E402

# Problem constants (hardcoded per the harness contract).
N_HEADS = 16
D_HEAD = 64
DEPTH = 12
LAMBDA_INIT = 0.8 - 0.6 * math.exp(-0.3 * DEPTH)
SCALING = 1.0 / math.sqrt(D_HEAD)
RMS_EPS = 1e-6
B, T = 4, 1024
CFAC = 1.0 - LAMBDA_INIT

N_CORES = 8
PAIRS = (B * N_HEADS) // N_CORES  # head-pairs per core = 8
BLK = 128
NJ = T // BLK  # 8 s/q tiles
VW = 132  # v tile row width: 128 vd + ones col + 3 pad
MAGIC = 0x5F3759DF  # fast-inverse-sqrt seed constant
LAG = 3  # lane-B stagger in steps


def _chunks(ext):
    """Split a q-extent into PSUM-bank chunks <=512, each >=256 when
    possible."""
    out = []
    while ext > 512:
        take = 512 if ext - 512 >= 256 or ext - 512 == 0 else ext - 256
        out.append(take)
        ext -= take
    if ext:
        out.append(ext)
    return out


def _kernel_body(tc, qk_ap, v_ap, lamn_ap, wv_ap, out_ap):
    nc = tc.nc
    f32 = mybir.dt.float32
    f16 = mybir.dt.float16
    i32 = mybir.dt.int32
    Exp = mybir.ActivationFunctionType.Exp

    with ExitStack() as ctx:
        const = ctx.enter_context(tc.tile_pool(name="const", bufs=1))
        qkp = ctx.enter_context(tc.tile_pool(name="qkp", bufs=1))
        vp = ctx.enter_context(tc.tile_pool(name="vp", bufs=1))
        pp = ctx.enter_context(tc.tile_pool(name="pp", bufs=2 * NJ + 2))
        scp = ctx.enter_context(tc.tile_pool(name="scp", bufs=2, space="PSUM"))
        yp = ctx.enter_context(tc.tile_pool(name="yp", bufs=2, space="PSUM"))
        zp = ctx.enter_context(tc.tile_pool(name="zp", bufs=NJ + 2))
        z2p = ctx.enter_context(tc.tile_pool(name="z2p", bufs=4))
        stp = ctx.enter_context(tc.tile_pool(name="stp", bufs=12))
        smp = ctx.enter_context(tc.tile_pool(name="smp", bufs=4))
        outp = ctx.enter_context(tc.tile_pool(name="outp", bufs=4))

        # -lambda per pair, broadcast across partitions (host-prepared).
        lamn_sb = const.tile([BLK, PAIRS], f32)
        nc.sync.dma_start(out=lamn_sb, in_=lamn_ap)
        wv_sb = None
        if wv_ap is not None:
            wv_sb = const.tile([BLK, BLK], f32)
            nc.sync.dma_start(out=wv_sb, in_=wv_ap)

        # Prefetch ALL inputs up-front in priority order: lane-group qk
        # first (gates the first matmul), then that group's v, then the
        # next group, etc. 2KB+ contiguous partition lines throughout.
        qq_tiles, kk_tiles, v_tiles = {}, {}, {}
        for p in range(PAIRS):
            qq_tiles[p] = qkp.tile([BLK, T], f16, name=f"qq{p}", tag=f"qq{p}")
            kk_tiles[p] = qkp.tile([BLK, T], f16, name=f"kk{p}", tag=f"kk{p}")
            if p < 2:
                # First group gates the pipeline start: partition-split the
                # loads so each ring carries a quarter of the descriptors.
                for lo, hi in ((0, 64), (64, BLK)):
                    nc.sync.dma_start(out=kk_tiles[p][lo:hi, :],
                                      in_=qk_ap[2 * p + 1][lo:hi, :])
                for lo, hi in ((0, 64), (64, BLK)):
                    nc.sync.dma_start(out=qq_tiles[p][lo:hi, :],
                                      in_=qk_ap[2 * p][lo:hi, :])
            else:
                nc.sync.dma_start(out=qq_tiles[p], in_=qk_ap[2 * p])
                nc.sync.dma_start(out=kk_tiles[p], in_=qk_ap[2 * p + 1])
            if p % 2 == 1:
                for pv in (p - 1, p):
                    v_tiles[pv] = vp.tile([BLK, NJ, VW], f16, name=f"v{pv}", tag=f"v{pv}")
                    if pv < 2:
                        for lo, hi in ((0, 64), (64, BLK)):
                            nc.sync.dma_start(out=v_tiles[pv][lo:hi], in_=v_ap[pv][lo:hi])
                    else:
                        nc.sync.dma_start(out=v_tiles[pv], in_=v_ap[pv])

        class Lane:
            """Per-head-pair tile state for interleaved two-lane emission."""

            def __init__(self, p):
                self.p = p
                self.qq_t = qq_tiles[p]
                self.kk_t = kk_tiles[p]
                self.v_t = v_tiles[p]
                self.stats = stp.tile([BLK, NJ], f32, tag="stats")
                self.pts = []
                self.zs = []

            def step(self, t):
                if t < NJ:
                    self.emit_qk_exp(t)
                if t >= 2 and t % 2 == 0:
                    self.emit_pv_epilogue(t // 2 - 1)
                if t == NJ:
                    self.finalize()

            def emit_qk_exp(lane, j):
                """QK^T + exp + diag mask for s-tile j -> P~ tile."""
                ext = T - BLK * j
                pt = pp.tile([BLK, 2, T], f16, name="pt", tag="pt")
                c0 = 0
                for cn in _chunks(ext):
                    sc = scp.tile([BLK, 2, 512], f32, tag="sc")
                    for h in range(2):
                        lhsT = lane.kk_t[64 * h : 64 * h + 64, BLK * j : BLK * j + BLK]
                        rhs = lane.qq_t[
                            64 * h : 64 * h + 64, BLK * j + c0 : BLK * j + c0 + cn
                        ]
                        # K=64 per head: pack the two heads into the top/bottom
                        # halves of the PE array - they run concurrently.
                        nc.tensor.matmul(
                            sc[:, h, 0:cn],
                            lhsT,
                            rhs,
                            start=True,
                            stop=True,
                            tile_position=(64 * h, 0),
                        )
                    nc.scalar.activation(
                        out=pt[:, :, c0 : c0 + cn],
                        in_=sc[:, :, 0:cn],
                        func=Exp,
                        scale=SCALING,
                    )
                    c0 += cn
                # zero the s>q upper triangle of the diagonal block in place
                # (single gpsimd op covering both heads).
                nc.gpsimd.affine_select(
                    out=pt[:, :, 0:BLK],
                    in_=pt[:, :, 0:BLK],
                    compare_op=AluOpType.is_ge,
                    fill=0.0,
                    base=0,
                    pattern=[[0, 2], [1, BLK]],
                    channel_multiplier=-1,
                )
                lane.pts.append(pt)

            def emit_pv_epilogue(lane, I):
                """PV accumulation + z epilogue for q-tiles 2I, 2I+1.

                Both q-tiles' Y live in one 2-bank PSUM tile so the
                batchable epilogue ops (reciprocal, sm, z-add) run once
                per supertile; col 128/384 of each bank = softmax denoms.
                """
                Yb = yp.tile([BLK, 2, 512], f32, tag="y")
                for ii in range(2):
                    i = 2 * I + ii
                    for jj in range(i + 1):
                        off = BLK * (i - jj)
                        for h in range(2):
                            nc.tensor.matmul(
                                Yb[:, ii, 256 * h : 256 * h + 129],
                                lane.pts[jj][:, h, off : off + BLK],
                                lane.v_t[:, jj, 0:129],
                                start=(jj == 0 and h == 0),
                                stop=(jj == i),
                                skip_group_check=True,
                            )

                # z_i = Y1_i - (lam * s1_i / s2_i) * Y2_i
                sm = smp.tile([BLK, 2, 2], f32, tag="sm")
                nc.vector.reciprocal(sm[:, :, 0:1], Yb[:, :, 384:385])
                nc.vector.scalar_tensor_tensor(
                    out=sm[:, :, 1:2],
                    in0=sm[:, :, 0:1],
                    scalar=lamn_sb[:, lane.p : lane.p + 1],
                    in1=Yb[:, :, 128:129],
                    op0=AluOpType.mult,
                    op1=AluOpType.mult,
                )
                z = zp.tile([BLK, 2, BLK], f16, tag="z")
                for ii in range(2):
                    nc.vector.tensor_scalar_mul(
                        z[:, ii, :], Yb[:, ii, 256:384], sm[:, ii, 1:2]
                    )
                nc.vector.tensor_tensor(
                    out=z, in0=z, in1=Yb[:, :, 0:128], op=AluOpType.add
                )
                z2 = z2p.tile([BLK, BLK], f16, tag="z2")
                for ii in range(2):
                    i = 2 * I + ii
                    nc.vector.scalar_tensor_tensor(
                        out=z2,
                        in0=z[:, ii, :],
                        scalar=1.0,
                        in1=z[:, ii, :],
                        op0=AluOpType.bypass,
                        op1=AluOpType.mult,
                        accum_out=lane.stats[:, i : i + 1],
                    )
                lane.zs.append(z)

            def finalize(lane):
                """rs = CFAC*rsqrt(mean+eps) via DVE fast-inverse-sqrt (no ACT
                tables), then per-pair output scale on gpsimd + one contiguous
                out DMA. Runs overlapped with the remaining pairs' compute."""
                a_ = 1.0 / (BLK * CFAC * CFAC)
                b_ = RMS_EPS / (CFAC * CFAC)
                x = stp.tile([BLK, NJ], f32, tag="x")
                nc.vector.tensor_scalar(
                    x, lane.stats, a_, b_, AluOpType.mult, AluOpType.add
                )
                h_t = stp.tile([BLK, NJ], f32, tag="h")
                nc.vector.tensor_scalar(h_t, x, 0.5, None, AluOpType.mult)
                yi = stp.tile([BLK, NJ], i32, tag="yi")
                nc.vector.tensor_scalar(
                    yi, x.bitcast(i32), 1, None, AluOpType.logical_shift_right
                )
                nc.vector.tensor_scalar(yi, yi, -1, None, AluOpType.bitwise_xor)
                nc.vector.tensor_scalar(yi, yi, MAGIC + 1, None, AluOpType.add)
                y = yi.bitcast(f32)
                s = stp.tile([BLK, NJ], f32, tag="s")
                for _ in range(2):
                    nc.vector.tensor_tensor(out=s, in0=y, in1=y, op=AluOpType.mult)
                    nc.vector.tensor_tensor(out=s, in0=s, in1=h_t, op=AluOpType.mult)
                    nc.vector.tensor_scalar(
                        s, s, -1.0, 1.5, AluOpType.mult, AluOpType.add
                    )
                    nc.vector.tensor_tensor(out=y, in0=y, in1=s, op=AluOpType.mult)
                o_t = outp.tile([BLK, NJ, BLK], f16, tag="o")
                for i in range(NJ):
                    nc.vector.tensor_scalar_mul(
                        o_t[:, i, :], lane.zs[i // 2][:, i % 2, :],
                        y[:, i : i + 1]
                    )
                    if wv_sb is not None:
                        nc.vector.tensor_tensor(
                            out=o_t[:, i, :], in0=o_t[:, i, :], in1=wv_sb,
                            op=AluOpType.mult,
                        )
                    if i == NJ // 2 - 1:
                        nc.sync.dma_start(out=out_ap[lane.p][:, 0 : NJ // 2],
                                          in_=o_t[:, 0 : NJ // 2])
                nc.sync.dma_start(out=out_ap[lane.p][:, NJ // 2 :],
                                  in_=o_t[:, NJ // 2 :])

        # Two staggered lanes per group: engines are strict in-order, so
        # interleaving two head-pairs (lane B lagging by LAG steps) keeps
        # independent work adjacent in each engine queue.
        assert PAIRS % 2 == 0
        for g in range(PAIRS // 2):
            lag = 1 if g == PAIRS // 2 - 1 else LAG
            laneA = Lane(2 * g)
            laneB = Lane(2 * g + 1)
            for t in range(NJ + 1 + lag):
                if t <= NJ:
                    laneA.step(t)
                if 0 <= t - lag <= NJ:
                    laneB.step(t - lag)


def build_program(pairs=PAIRS, apply_weight=False, num_devices=N_CORES):
    global PAIRS
    saved = PAIRS
    PAIRS = pairs
    try:
        nc = bacc.Bacc(
            "TRN2", target_bir_lowering=False, debug=False, num_devices=num_devices
        )
        qk_d = nc.dram_tensor(
            "qk", [2 * pairs, BLK, T], mybir.dt.float16, kind="ExternalInput"
        )
        v_d = nc.dram_tensor(
            "v", [pairs, BLK, NJ, VW], mybir.dt.float16, kind="ExternalInput"
        )
        lamn_d = nc.dram_tensor(
            "lamn", [BLK, pairs], mybir.dt.float32, kind="ExternalInput"
        )
        wv_d = None
        if apply_weight:
            wv_d = nc.dram_tensor(
                "wv", [BLK, BLK], mybir.dt.float32, kind="ExternalInput"
            )
        out_d = nc.dram_tensor(
            "out", [pairs, BLK, NJ, BLK], mybir.dt.float16, kind="ExternalOutput"
        )
        with tile.TileContext(nc) as tc:
            _kernel_body(
                tc,
                qk_d.ap(),
                v_d.ap(),
                lamn_d.ap(),
                wv_d.ap() if wv_d is not None else None,
                out_d.ap(),
            )
        nc.compile()
        return nc
    finally:
        PAIRS = saved


def make_in_maps(q, k, v, lambda_q1, lambda_k1, lambda_q2, lambda_k2, rms_weight):
    """Host-side shard + layout prep. Returns (in_maps, apply_weight)."""
    q = np.ascontiguousarray(np.asarray(q, np.float32).transpose(0, 1, 3, 2))
    k = np.ascontiguousarray(np.asarray(k, np.float32).transpose(0, 1, 3, 2))
    v = np.asarray(v, np.float32)
    lq1 = np.asarray(lambda_q1, np.float64)
    lk1 = np.asarray(lambda_k1, np.float64)
    lq2 = np.asarray(lambda_q2, np.float64)
    lk2 = np.asarray(lambda_k2, np.float64)
    lam1 = np.exp(np.sum(lq1 * lk1, axis=-1))
    lam2 = np.exp(np.sum(lq2 * lk2, axis=-1))
    lam = (lam1 - lam2 + LAMBDA_INIT).astype(np.float32)  # [N_HEADS]
    w = np.asarray(rms_weight, np.float32)
    apply_weight = not np.all(w == 1.0)

    in_maps = []
    for c in range(N_CORES):
        qk_c = np.empty((2 * PAIRS, BLK, T), np.float16)
        v_c = np.zeros((PAIRS, BLK, NJ, VW), np.float16)
        lamn_c = np.empty((BLK, PAIRS), np.float32)
        for p in range(PAIRS):
            g = c * PAIRS + p
            b, h = divmod(g, N_HEADS)
            # [2p] = stacked q~ of both score heads, [2p+1] = stacked k~.
            qk_c[2 * p, 0:64] = q[b, 2 * h]
            qk_c[2 * p, 64:128] = q[b, 2 * h + 1]
            qk_c[2 * p + 1, 0:64] = k[b, 2 * h]
            qk_c[2 * p + 1, 64:128] = k[b, 2 * h + 1]
            # v[b,h] [T, 128] -> [BLK, NJ, 128] with s = 128*jj + q0
            v_c[p, :, :, 0:128] = (
                v[b, h].reshape(NJ, BLK, BLK).transpose(1, 0, 2).astype(np.float16)
            )
            v_c[p, :, :, 128] = 1.0
            lamn_c[:, p] = -lam[h]
        m = {"qk": qk_c, "v": v_c, "lamn": lamn_c}
        if apply_weight:
            m["wv"] = np.broadcast_to(w[None, :], (BLK, BLK)).copy()
        in_maps.append(m)
    return in_maps, apply_weight


def kernel(q, k, v, mask, lambda_q1, lambda_k1, lambda_q2, lambda_k2,
           rms_weight, flash_attn=0, _trace=False, _nc_cache={}):
    in_maps, apply_weight = make_in_maps(
        q, k, v, lambda_q1, lambda_k1, lambda_q2, lambda_k2, rms_weight
    )
    if apply_weight not in _nc_cache:
        _nc_cache[apply_weight] = build_program(apply_weight=apply_weight)
    nc = _nc_cache[apply_weight]
    res = bass_utils.run_bass_kernel_spmd(
        nc, in_maps, core_ids=list(range(N_CORES)), trace=_trace
    )
    out = np.empty((B, N_HEADS, T, 2 * D_HEAD), np.float32)
    for c in range(N_CORES):
        oc = res.results[c]["out"].astype(np.float32)  # [PAIRS, BLK, NJ, BLK]
        for p in range(PAIRS):
            g = c * PAIRS + p
            b, h = divmod(g, N_HEADS)
            out[b, h] = oc[p].transpose(1, 0, 2).reshape(T, 2 * D_HEAD)
    if _trace:
        kernel._last_exec_time_ns = res.exec_time_ns
        kernel._last_results = res
    return out
